# revision 1
# baseline (speedup 1.0000x reference)
"""Trainium2 distributed kernel for nn_AttentionFusion (BEV temporal+spatial attention).

Full computation on device across 8 NeuronCores, zero cross-core communication.

Sharding: 2x4 grid of core blocks (50x25 grid pixels per core). Each core
processes its block PLUS a 2-pixel halo (54x29 local region, out-of-grid
pixels zero) so the spatial neighbor windows are always core-local
(redundant temporal compute on the halo instead of a collective).

Per core:
  phase 1 (13 chunks of 128 px): temporal attention. x arrives channel-major
    (host pre-transposed bf16). h_t is never materialized: tWo is composed
    into the spatial projections on the host; k/v biases cancel or fold
    (softmax shift invariance + sum(p)=1). Writes a local kv table
    [1664, 768] = [k padded to 64/head | v] to DRAM.
  phase 2 (25 chunks of 10x5 queries): spatial window attention. The 126-px
    (14x9) window k is fetched channel-major with one transpose-mode
    dma_gather, v pixel-major with a second gather; scores/ctx are dense
    per-head matmuls, masked by a host-precomputed band mask. exp without
    max-subtraction (scores are tiny); softmax denominator via a ones-column
    matmul fused into the ctx pass.
  phase 3 (10 chunks): output projection.

Self-contained: only needs the container toolchain at /opt/trn_rl_repo.
"""

import math
import os
import sys

import numpy as np

sys.path.insert(0, "/opt/trn_rl_repo")

import ml_dtypes  # noqa: E402

import concourse.bass as bass  # noqa: E402
import concourse.bacc as bacc  # noqa: E402
import concourse.mybir as mybir  # noqa: E402
import concourse.tile as tile  # noqa: E402

F32 = mybir.dt.float32
FP8 = mybir.dt.float8e4
BF16 = mybir.dt.bfloat16
I16 = mybir.dt.int16
AX = mybir.AxisListType
ALU = mybir.AluOpType
ACTF = mybir.ActivationFunctionType

# Problem constants
N_FULL = 10000
GRID = 100
T = 5
C = 256
NH = 8
DK = 32
CORES = 8
CR, CC_ = 2, 4             # core grid 2 x 4
BR, BC = 50, 25            # block rows/cols per core
NLOC = BR * BC             # 1250 real pixels per core
HR, HC = BR + 4, BC + 4    # 54 x 29 local region (with halo)
NH_PIX = HR * HC           # 1566
NP = 128
G = (NH_PIX + NP - 1) // NP        # 13 projection chunks
NPAD = G * NP                      # 1664
GO = 10                            # output-projection chunks
NPO = NLOC // GO                   # 125
# spatial chunks: 10 rows x 5 cols of queries
SQR, SQC = 10, 5
NS_R, NS_C = BR // SQR, BC // SQC  # 5 x 5 = 25
NS = NS_R * NS_C
NQ = SQR * SQC             # 50
WR, WC = SQR + 4, SQC + 4  # 14 x 9
NW = WR * WC               # 126
KROW = 384                 # k section: 3 heads per 128-col group at offsets {0,32,64}
# head slot hh (in kq32 tables) -> standard head: hh = 3*(h%3) + h//3
PERM = [0, 3, 6, 1, 4, 7, 2, 5]
VROW = 256
ROW = KROW + VROW          # 768
SEGR = 34                  # rows per overlap segment (2 segments: rows [0,34), [20,54))
SEGP = SEGR * HC           # 986 pixels per segment

_CACHE = {}


def _bf16(a):
    return np.asarray(a, dtype=ml_dtypes.bfloat16)


def _pad_cols(w):
    """[C, 256] -> [C, 384]: head h (32 cols) at 128*(h//3) + 32*(h%3)."""
    w = w.reshape(-1, NH * DK)
    out = np.zeros((w.shape[0], KROW), np.float32)
    for h in range(NH):
        base = 128 * (h // 3) + 32 * (h % 3)
        out[:, base : base + DK] = w[:, DK * h : DK * (h + 1)]
    return out


def _build_graph():
    nc = bacc.Bacc(
        "TRN2",
        target_bir_lowering=False,
        debug=False,
        enable_asserts=False,
        num_devices=CORES,
    )

    # ---------------- I/O ----------------
    x_d = nc.dram_tensor("x", [G, 128, 2 * T * NP], FP8, kind="ExternalInput")
    w_d = {
        "wkv_t": nc.dram_tensor("wkv_t", [2, 128, 2 * C], BF16, kind="ExternalInput"),
        "wq_t": nc.dram_tensor("wq_t", [2, 128, C], BF16, kind="ExternalInput"),
        # spatial q projection, transposed output layout, head-padded [2,128,512]
        "wq_e": nc.dram_tensor("wq_e", [2, 128, KROW], BF16, kind="ExternalInput"),
        # fused (k_padded | v) projection [2, 128, 768]
        "wkv_e": nc.dram_tensor("wkv_e", [2, 128, ROW], BF16, kind="ExternalInput"),
        "wo_s": nc.dram_tensor("wo_s", [2, 128, C], BF16, kind="ExternalInput"),
    }
    b_d = {
        "bq_t": nc.dram_tensor("bq_t", [1, C], BF16, kind="ExternalInput"),
        "bq_e": nc.dram_tensor("bq_e", [1, KROW], BF16, kind="ExternalInput"),
        "bo_e": nc.dram_tensor("bo_e", [1, C], BF16, kind="ExternalInput"),
    }
    ident_d = nc.dram_tensor("ident", [128, 128], BF16, kind="ExternalInput")
    ones1_d = nc.dram_tensor("ones1", [1, 128], BF16, kind="ExternalInput")
    masks_d = nc.dram_tensor("masks", [NW, NS * NQ], BF16, kind="ExternalInput")
    out_d = nc.dram_tensor("out", [NLOC, C], BF16, kind="ExternalOutput")

    with tile.TileContext(nc) as tc:
        with (
            tc.tile_pool(name="const", bufs=1) as cpool,
            tc.tile_pool(name="dram", bufs=1, space="DRAM") as dpool,
            tc.tile_pool(name="sb", bufs=4) as sb,
            tc.tile_pool(name="pkv", bufs=2, space="PSUM") as pkv,
            tc.tile_pool(name="pproj", bufs=1, space="PSUM") as pproj,
            tc.tile_pool(name="ptp", bufs=1, space="PSUM") as ptp,
            tc.tile_pool(name="psc", bufs=1, space="PSUM") as psc,
            tc.tile_pool(name="pcx", bufs=1, space="PSUM") as pcx,
        ):
            v_dram = [
                dpool.tile([SEGP, VROW], BF16, tag=f"v_dram{i}", name=f"v_dram{i}")
                for i in range(2)
            ]

            # ---------- constants ----------
            w_sb = {}
            for n, d in w_d.items():
                t_ = cpool.tile([128, 2, d.shape[2]], BF16, tag=f"w_{n}")
                nc.sync.dma_start(t_[:], d.ap().rearrange("a p c -> p a c"))
                w_sb[n] = t_
            b_sb = {}
            for n, d in b_d.items():
                t_ = cpool.tile([1, d.shape[1]], BF16, tag=f"b_{n}")
                nc.sync.dma_start(t_[:], d.ap())
                b_sb[n] = t_
            ident = cpool.tile([128, 128], BF16, tag="ident")
            nc.sync.dma_start(ident[:], ident_d.ap())
            ones1 = cpool.tile([1, 128], BF16, tag="ones1")
            nc.sync.dma_start(ones1[:], ones1_d.ap())
            onesw = cpool.tile([128, 1], BF16, tag="onesw")
            nc.vector.memset(onesw[:], 1.0)
            masks = cpool.tile([128, NS, NQ], BF16, tag="masks")
            nc.sync.dma_start(
                masks[0:NW, :, :], masks_d.ap().rearrange("w (s q) -> w s q", s=NS)
            )
            kq32 = [
                cpool.tile([32, NH, 2, SEGP], BF16, tag=f"kq32_{i}", name=f"kq32_{i}")
                for i in range(2)
            ]
            cT_all = cpool.tile([128, 2, NLOC], BF16, tag="cT_all")

            def bias_mm(psum_t, b_key, n_out, rows):
                nc.tensor.matmul(
                    psum_t,
                    ones1[0:1, 0:rows],
                    b_sb[b_key][0:1, 0:n_out],
                    start=False,
                    stop=True,
                )

            # ================= PHASE 1: temporal =================
            for g in range(G):
                xg = sb.tile([128, 2, T, NP], BF16, tag="xg")
                nc.gpsimd.dma_start(
                    xg[:], x_d.ap()[g].rearrange("p (a t n) -> p a t n", a=2, t=T)
                )

                kv_sb = sb.tile([NP, T, 2 * C], BF16, tag="kv_sb")
                for t0 in range(0, T, 2):
                    nt = min(2, T - t0)
                    kvp = pkv.tile([NP, 2, 2 * C], F32, tag="kvp")
                    for dt_ in range(nt):
                        for cc in range(2):
                            nc.tensor.matmul(
                                kvp[:, dt_, :],
                                xg[:, cc, t0 + dt_, :],
                                w_sb["wkv_t"][:, cc, :],
                                start=(cc == 0),
                                stop=(cc == 1),
                            )
                    nc.scalar.copy(
                        kv_sb[:, t0 : t0 + nt, :], kvp[:, 0:nt, :]
                    )
                k_sb = kv_sb[:, :, 0:C]
                v_sb = kv_sb[:, :, C : 2 * C]

                qp = pkv.tile([NP, 2, 2 * C], F32, tag="kvp", name="qp")[:, 0, 0:C]
                for cc in range(2):
                    nc.tensor.matmul(
                        qp[:], xg[:, cc, T - 1, :], w_sb["wq_t"][:, cc, :],
                        start=(cc == 0), stop=False,
                    )
                bias_mm(qp[:], "bq_t", C, NP)
                q_sb = sb.tile([NP, C], BF16, tag="q_sb")
                nc.scalar.copy(q_sb[:], qp[:])

                # scores over t (no max subtraction: |s| < 1)
                prod = sb.tile([NP, T, NH, DK], BF16, tag="prod")
                nc.vector.tensor_mul(
                    prod[:],
                    k_sb.rearrange("p t (h d) -> p t h d", h=NH),
                    q_sb[:].rearrange("p (h d) -> p h d", h=NH)
                    .unsqueeze(1)
                    .broadcast_to((NP, T, NH, DK)),
                )
                s_t = sb.tile([NP, T, NH], BF16, tag="s_t")
                with nc.allow_low_precision(reason="temporal scores bf16"):
                    nc.vector.tensor_reduce(s_t[:], prod[:], axis=AX.X, op=ALU.add)
                es = sb.tile([NP, T, NH], F32, tag="es")
                nc.scalar.activation(es[:], s_t[:], ACTF.Exp)
                tsum = sb.tile([NP, NH], F32, tag="tsum")
                nc.vector.tensor_reduce(
                    tsum[:], es[:].rearrange("p t h -> p h t"), axis=AX.X, op=ALU.add
                )
                rinv = sb.tile([NP, NH], F32, tag="trinv")
                nc.vector.reciprocal(rinv[:], tsum[:])
                p_t = sb.tile([NP, T, NH], BF16, tag="p_t")
                nc.vector.tensor_mul(
                    p_t[:], es[:], rinv[:].unsqueeze(1).broadcast_to((NP, T, NH))
                )

                wv = sb.tile([NP, T, C], BF16, tag="wv")
                nc.vector.tensor_mul(
                    wv[:].rearrange("p t (h d) -> p t h d", h=NH),
                    v_sb.rearrange("p t (h d) -> p t h d", h=NH),
                    p_t[:].unsqueeze(3).broadcast_to((NP, T, NH, DK)),
                )
                c1 = sb.tile([NP, 2, C], BF16, tag="c1")
                nc.vector.tensor_add(c1[:], wv[:, 0:2, :], wv[:, 2:4, :])
                c2 = sb.tile([NP, C], BF16, tag="c2")
                nc.vector.tensor_add(c2[:], c1[:, 0, :], c1[:, 1, :])
                ctx = sb.tile([NP, C], BF16, tag="ctx")
                nc.vector.tensor_add(ctx[:], c2[:], wv[:, 4, :])

                # ctxT (channel-major)
                ctp = ptp.tile([128, 2, NP], BF16, tag="ctp")
                for cc in range(2):
                    nc.tensor.transpose(
                        ctp[:, cc, :], ctx[:, cc * 128 : (cc + 1) * 128],
                        ident[0:NP, 0:NP],
                    )
                ctxT = sb.tile([128, 2, NP], BF16, tag="ctxT")
                nc.scalar.copy(ctxT[:], ctp[:])

                # q'T via transposed projection (skipped for the last chunk:
                # rows >= 53 are halo/pad, their q' is never read)
                need_q = g * NP < (HR - 2) * HC - 2 * HC + 0 or True
                need_q = g * NP < 1535
                qtp = pproj.tile([128, 3, NP], F32, tag="proj", name="qtp")
                for grp in (range(3) if need_q else []):
                    for cc in range(2):
                        nc.tensor.matmul(
                            qtp[:, grp, :],
                            w_sb["wq_e"][:, cc, 128 * grp : 128 * (grp + 1)],
                            ctxT[:, cc, :],
                            start=(cc == 0),
                            stop=False,
                        )
                    nc.tensor.matmul(
                        qtp[:, grp, :],
                        b_sb["bq_e"][0:1, 128 * grp : 128 * (grp + 1)],
                        ones1[0:1, 0:NP],
                        start=False,
                        stop=True,
                    )
                kq_tmp = sb.tile([128, 3, 2, NP], BF16, tag="kq_tmp")
                if need_q:
                    nc.vector.tensor_copy(kq_tmp[:, :, 1, :], qtp[:])

                # k1 transposed projection
                ktp = pproj.tile([128, 3, NP], F32, tag="proj", name="ktp")
                for grp in range(3):
                    for cc in range(2):
                        nc.tensor.matmul(
                            ktp[:, grp, :],
                            w_sb["wkv_e"][:, cc, 128 * grp : 128 * (grp + 1)],
                            ctxT[:, cc, :],
                            start=(cc == 0),
                            stop=(cc == 1),
                        )
                nc.scalar.copy(kq_tmp[:, :, 0, :], ktp[:])
                # head regroup: partitions [32m:32m+32) -> head slots [3m:3m+3),
                # scattered into the overlap segments for phase-1/2 pipelining
                glo, ghi = g * NP, min(NH_PIX, (g + 1) * NP)
                for seg in range(2):
                    slo = seg * 20 * HC
                    shi = slo + SEGP
                    lo, hi = max(glo, slo), min(ghi, shi)
                    if lo >= hi:
                        continue
                    nkq = 2 if need_q else 1
                    for m in range(3):
                        nc.sync.dma_start(
                            kq32[seg][:, 3 * m : min(3 * m + 3, NH), 0:nkq,
                                      lo - slo : hi - slo],
                            kq_tmp[32 * m : 32 * m + 32,
                                   0 : (3 if m < 2 else 2), 0:nkq,
                                   lo - glo : hi - glo],
                        )
                vp = pproj.tile([NP, VROW], F32, tag="proj", name="vp")
                for cc in range(2):
                    nc.tensor.matmul(
                        vp[:], ctxT[:, cc, :], w_sb["wkv_e"][:, cc, KROW:ROW],
                        start=(cc == 0), stop=(cc == 1),
                    )
                v1_sb = sb.tile([NP, VROW], BF16, tag="v1_sb")
                nc.scalar.copy(v1_sb[:], vp[:])
                for seg in range(2):
                    slo = seg * 20 * HC
                    shi = slo + SEGP
                    lo, hi = max(glo, slo), min(ghi, shi)
                    if lo < hi:
                        nc.gpsimd.dma_start(
                            v_dram[seg][lo - slo : hi - slo, :],
                            v1_sb[lo - glo : hi - glo, :],
                        )

            PH = os.environ.get("KERNEL_PHASES", "123")
            # ================= PHASE 2: spatial =================
            for s in (range(NS) if "2" in PH else []):
                r0, c0 = (s // NS_C) * SQR, (s % NS_C) * SQC
                seg = 0 if r0 <= 20 else 1
                rs = r0 - 20 * seg
                vw = sb.tile([NW, VROW], BF16, tag="vw")
                nc.gpsimd.dma_start(
                    vw[:],
                    v_dram[seg][:].rearrange("(r c) x -> r c x", r=SEGR)[
                        rs : rs + WR, c0 : c0 + WC, :
                    ],
                )
                kqv = kq32[seg][:].rearrange("p h t (r c) -> p h t r c", r=SEGR)
                kTw = sb.tile([32, NH, NW], BF16, tag="kTw")
                nc.vector.tensor_copy(
                    kTw[:].rearrange("p h (r c) -> p h r c", r=WR),
                    kqv[:, :, 0, rs : rs + WR, c0 : c0 + WC],
                )
                qw = sb.tile([32, NH, NQ], BF16, tag="qw")
                nc.vector.tensor_copy(
                    qw[:].rearrange("p h (r c) -> p h r c", r=SQR),
                    kqv[:, :, 1, 2 + rs : 2 + rs + SQR, 2 + c0 : 2 + c0 + SQC],
                )

                LVL = int(os.environ.get("KERNEL_P2LVL", "9"))
                if LVL < 1:
                    continue
                sc = psc.tile([NW, NH, NQ], F32, tag="sc")
                for h in range(NH):
                    nc.tensor.matmul(
                        sc[:, h, :],
                        kTw[:, h, :],
                        qw[:, h, :],
                        start=True,
                        stop=True,
                    )
                if LVL < 2:
                    continue
                E = sb.tile([NW, NH, NQ], BF16, tag="E")
                nc.scalar.activation(E[:], sc[:], ACTF.Exp)
                E2 = sb.tile([NW, NH, NQ], BF16, tag="E2")
                nc.gpsimd.tensor_mul(
                    E2[:],
                    E[:],
                    masks[0:NW, s, :].unsqueeze(1).broadcast_to((NW, NH, NQ)),
                )

                if LVL < 3:
                    continue
                cx = pcx.tile([NQ, NH, DK + 1], F32, tag="cx")
                for h in range(NH):
                    nc.tensor.matmul(
                        cx[:, h, 0:DK],
                        E2[:, h, :],
                        vw[0:NW, DK * PERM[h] : DK * PERM[h] + DK],
                        start=True,
                        stop=True,
                    )
                    nc.tensor.matmul(
                        cx[:, h, DK : DK + 1],
                        E2[:, h, :],
                        onesw[0:NW, :],
                        start=True,
                        stop=True,
                    )
                srinv = sb.tile([NQ, NH], F32, tag="srinv")
                nc.vector.reciprocal(srinv[:], cx[:, :, DK])
                ctxn = sb.tile([NQ, C], BF16, tag="ctxn")
                nc.vector.tensor_mul(
                    ctxn[:].rearrange("q (h d) -> q h d", h=NH),
                    cx[:, :, 0:DK],
                    srinv[:].unsqueeze(2).broadcast_to((NQ, NH, DK)),
                )
                if LVL < 4:
                    continue
                ntp = ptp.tile([128, 2, NQ], BF16, tag="ctp")
                for cc in range(2):
                    nc.tensor.transpose(
                        ntp[:, cc, :], ctxn[:, cc * 128 : (cc + 1) * 128],
                        ident[0:NQ, 0:NQ],
                    )
                csel = cT_all[:, :, :].rearrange(
                    "p a (r c) -> p a r c", r=BR
                )[:, :, r0 : r0 + SQR, c0 : c0 + SQC]
                nc.scalar.copy(
                    csel[:],
                    ntp[:].rearrange("p a (r c) -> p a r c", r=SQR),
                )

            # ================= PHASE 3: output proj =================
            if "3" not in PH:
                zz = sb.tile([NPO, C], BF16, tag="o_sb", name="zz")
                nc.vector.memset(zz[:], 0.0)
                nc.gpsimd.dma_start(out_d.ap()[0:NPO, :], zz[:])
            for g in (range(GO) if "3" in PH else []):
                op = pkv.tile([NPO, 2, 2 * C], F32, tag="kvp", name="op")[:, 0, 0:C]
                for cc in range(2):
                    nc.tensor.matmul(
                        op[:], cT_all[:, cc, g * NPO : (g + 1) * NPO],
                        w_sb["wo_s"][:, cc, :],
                        start=(cc == 0), stop=False,
                    )
                bias_mm(op[:], "bo_e", C, NPO)
                o_sb = sb.tile([NPO, C], BF16, tag="o_sb")
                nc.scalar.copy(o_sb[:], op[:])
                nc.gpsimd.dma_start(out_d.ap()[g * NPO : (g + 1) * NPO, :], o_sb[:])

    nc.compile()
    return nc


def _prep_weights(inputs):
    """Host-side weight transforms (all small)."""
    scale = 1.0 / math.sqrt(DK)
    f = lambda k: np.asarray(inputs[k], np.float32)
    tWq, tbq = f("t_Wq") * scale, f("t_bq") * scale
    tWk = f("t_Wk")
    tWv, tbv = f("t_Wv"), f("t_bv")
    tWo, tbo = f("t_Wo"), f("t_bo")
    sWq, sbq = f("s_Wq"), f("s_bq")
    sWk = f("s_Wk")
    sWv, sbv = f("s_Wv"), f("s_bv")
    sWo, sbo = f("s_Wo"), f("s_bo")

    hb = tbv @ tWo + tbo                    # constant part of h_t
    Wq_eff = (tWo @ sWq) * scale
    bq_eff = (hb @ sWq + sbq) * scale
    Wk_eff = tWo @ sWk                      # k bias dropped (softmax-invariant)
    Wv_eff = tWo @ sWv
    cv = hb @ sWv + sbv                     # constant part of v1
    bo_eff = sbo + cv @ sWo

    cm = {
        "wkv_t": _bf16(np.concatenate([tWk, tWv], axis=1).reshape(2, 128, 2 * C)),
        "wq_t": _bf16(tWq.reshape(2, 128, C)),
        "wq_e": _bf16(_pad_cols(Wq_eff).reshape(2, 128, KROW)),
        "wkv_e": _bf16(
            np.concatenate([_pad_cols(Wk_eff), Wv_eff], axis=1).reshape(2, 128, ROW)
        ),
        "wo_s": _bf16(
            sWo.reshape(NH, DK, C)[PERM].reshape(2, 128, C)
        ),
        "bq_t": _bf16(tbq.reshape(1, C)),
        "bq_e": _bf16(_pad_cols(bq_eff).reshape(1, KROW)),
        "bo_e": _bf16(bo_eff.reshape(1, C)),
        "ident": _bf16(np.eye(128, dtype=np.float32)),
        "ones1": _bf16(np.ones((1, 128), np.float32)),
    }
    return cm


def _prep_geometry():
    """Per-core gather indices (local coords) and window masks."""
    masks = np.zeros((CORES, NW, NS * NQ), np.float32)
    for R in range(CR):
        for C4 in range(CC_):
            core = R * CC_ + C4
            for s in range(NS):
                r0, c0 = (s // NS_C) * SQR, (s % NS_C) * SQC
                gr0, gc0 = BR * R + r0, BC * C4 + c0
                wr = gr0 - 2 + np.arange(WR)          # global window rows
                wc = gc0 - 2 + np.arange(WC)
                valid = (wr[:, None] >= 0) & (wr[:, None] < GRID) & \
                        (wc[None, :] >= 0) & (wc[None, :] < GRID)
                qr = gr0 + np.arange(SQR)
                qc = gc0 + np.arange(SQC)
                qrc = np.clip(qr, 2, GRID - 3)
                qcc = np.clip(qc, 2, GRID - 3)
                mrow = (np.abs(wr[:, None] - qrc[None, :]) <= 2)
                mcol = (np.abs(wc[:, None] - qcc[None, :]) <= 2)
                m = (mrow[:, None, :, None] & mcol[None, :, None, :] &
                     valid[:, :, None, None])
                masks[core, :, s * NQ : (s + 1) * NQ] = m.reshape(NW, NQ)
    return _bf16(masks)


def _prep_x(x):
    """x [10000, 5, 256] f32 -> per-core halo-extended channel-major bf16
    chunks [8, G, 128, 2*T*128]."""
    xb = np.asarray(x, np.float32).astype(ml_dtypes.float8_e4m3).reshape(
        GRID, GRID, T, C
    )
    xp = np.zeros((GRID + 4, GRID + 4, T, C), dtype=xb.dtype)
    xp[2 : 2 + GRID, 2 : 2 + GRID] = xb
    out = np.zeros((CORES, G, 128, 2 * T * NP), dtype=xb.dtype)
    for R in range(CR):
        for C4 in range(CC_):
            core = R * CC_ + C4
            blk = xp[BR * R : BR * R + HR, BC * C4 : BC * C4 + HC]  # [54,29,T,C]
            flat = blk.reshape(NH_PIX, T, C)
            flat = np.concatenate(
                [flat, np.zeros((NPAD - NH_PIX, T, C), dtype=xb.dtype)], axis=0
            )
            v = flat.reshape(G, NP, T, 2, 128)
            v = v.transpose(0, 4, 3, 2, 1)      # (g, ch, cc, t, px)
            out[core] = v.reshape(G, 128, 2 * T * NP)
    return out


def _unprep_out(res_list):
    """[8][1250, 256] bf16 -> [10000, 1, 256] f32 global row-major."""
    o = np.stack([np.asarray(r) for r in res_list], axis=0).astype(np.float32)
    v = o.reshape(CR, CC_, BR, BC, C)
    v = v.transpose(0, 2, 1, 3, 4)
    return np.ascontiguousarray(v.reshape(N_FULL, 1, C))


def _make_in_maps(inputs):
    cm = _prep_weights(inputs)
    if "geom" not in _CACHE:
        _CACHE["geom"] = _prep_geometry()
    masks = _CACHE["geom"]
    X = _prep_x(inputs["x"])
    in_maps = []
    for c in range(CORES):
        m = dict(cm)
        m["x"] = X[c]
        m["masks"] = masks[c]
        in_maps.append(m)
    return in_maps


def _get_runner(nc):
    """Build (once) and cache a jitted shard_map callable for the NEFF.

    run_bass_kernel_spmd re-traces and re-jits on every call; caching the
    callable drops warm-call dispatch to the PJRT execute + transfers.
    """
    if "runner" in _CACHE:
        return _CACHE["runner"]
    import jax
    import numpy as jnp_np  # noqa
    from jax.sharding import Mesh, PartitionSpec
    from jax.experimental.shard_map import shard_map
    import concourse.mybir as mb
    from concourse import bass2jax

    bass2jax.install_neuronx_cc_hook()

    in_names, out_names, out_avals, zero_shapes = [], [], [], []
    partition_name = (
        nc.partition_id_tensor.name if nc.partition_id_tensor else None
    )
    for alloc in nc.m.functions[0].allocations:
        if not isinstance(alloc, mb.MemoryLocationSet):
            continue
        name = alloc.memorylocations[0].name
        if alloc.kind == "ExternalInput":
            if name != partition_name:
                in_names.append(name)
        elif alloc.kind == "ExternalOutput":
            shape = tuple(alloc.tensor_shape)
            dtype = mb.dt.np(alloc.dtype)
            out_names.append(name)
            out_avals.append(jax.core.ShapedArray(shape, dtype))
            zero_shapes.append((shape, dtype))
    n_params = len(in_names)
    all_names = list(in_names) + list(out_names)
    if partition_name is not None:
        all_names.append(partition_name)
    donate = tuple(range(n_params, n_params + len(out_names)))

    def _body(*args):
        operands = list(args)
        if partition_name is not None:
            operands.append(bass2jax.partition_id_tensor())
        outs = bass2jax._bass_exec_p.bind(
            *operands,
            out_avals=tuple(out_avals),
            in_names=tuple(all_names),
            out_names=tuple(out_names),
            lowering_input_output_aliases=(),
            sim_require_finite=True,
            sim_require_nnan=True,
            nc=nc,
        )
        return tuple(outs)

    devices = jax.devices()[:CORES]
    mesh = Mesh(np.asarray(devices), ("core",))
    in_specs = (PartitionSpec("core"),) * (n_params + len(out_names))
    out_specs = (PartitionSpec("core"),) * len(out_names)
    sharded = jax.jit(
        shard_map(_body, mesh=mesh, in_specs=in_specs, out_specs=out_specs,
                  check_rep=False),
        donate_argnums=donate, keep_unused=True,
    )

    zfns = [
        jax.jit(
            lambda s=s, dt=dt: jax.numpy.zeros((CORES * s[0], *s[1:]), dt),
            out_shardings=jax.sharding.NamedSharding(mesh, PartitionSpec("core")),
        )
        for s, dt in zero_shapes
    ]
    in_shard = jax.sharding.NamedSharding(mesh, PartitionSpec("core"))

    def run(concat_in):
        args = []
        for n in in_names:
            v = concat_in[n]
            if isinstance(v, tuple):      # (digest, np array): device-cacheable
                key = ("dev", n, v[0])
                if key not in _CACHE:
                    _CACHE[key] = jax.device_put(v[1], in_shard)
                args.append(_CACHE[key])
            else:
                args.append(v)
        zeros = [zf() for zf in zfns]
        outs = sharded(*args, *zeros)
        return {n: outs[i] for i, n in enumerate(out_names)}

    _CACHE["runner"] = run
    return run


def _weights_digest(inputs):
    import hashlib
    h = hashlib.blake2b(digest_size=16)
    for k in sorted(inputs):
        if k not in ("x",):
            h.update(np.ascontiguousarray(inputs[k]).tobytes())
    return h.hexdigest()


def _make_concat_inputs(inputs):
    """Concatenated-along-core-axis input arrays for the cached runner.
    Weight/mask entries are (digest, array) tuples so the runner can keep
    them device-resident across calls."""
    dig = _weights_digest(inputs)
    cm = _prep_weights(inputs)
    if "geom" not in _CACHE:
        _CACHE["geom"] = _prep_geometry()
    masks = _CACHE["geom"]
    X = _prep_x(inputs["x"])
    cat = {}
    for k, v in cm.items():
        full = np.broadcast_to(v, (CORES,) + v.shape).reshape(
            (CORES * v.shape[0],) + v.shape[1:]
        )
        cat[k] = (dig, full)
    cat["x"] = X.reshape(CORES * G, 128, 2 * T * NP)
    cat["masks"] = ("geom", masks.reshape(CORES * NW, NS * NQ))
    return cat


def kernel(**inputs):
    if "nc" not in _CACHE:
        _CACHE["nc"] = _build_graph()
    nc = _CACHE["nc"]
    run = _get_runner(nc)
    cat = _make_concat_inputs(inputs)
    import time as _time
    t0 = _time.perf_counter()
    outs = run(cat)
    out_np = np.asarray(outs["out"])
    _CACHE["last_device_ns"] = (_time.perf_counter() - t0) * 1e9
    o = out_np.reshape(CORES, NLOC, C).astype(np.float32)
    v = o.reshape(CR, CC_, BR, BC, C).transpose(0, 2, 1, 3, 4)
    return np.ascontiguousarray(v.reshape(N_FULL, 1, C))



# revision 7
# speedup vs baseline: 1.0017x; 1.0017x over previous
"""Trainium2 distributed kernel for nn_AttentionFusion — v2.

Channel-major temporal attention, fp8 DoubleRow matmuls, PE score
reduction via block-ones matmuls, spatial mask folded into score PSUM,
direct strided-AP windows (no head-regroup scatter).

Per core: 2x4 grid of 50x25 blocks + 2-px halo (54x29 local = 1566 px,
padded to 13 chunks of 128).

Phase 1 (13 chunks): kT/qT ch-major + vPM pixel-major via fp8 DoubleRow;
scores = blockones matmuls over DVE products; softmax pixel-major;
ctx = p-weighted v (Pool); spatial projections from fp8 ctxT.
Phase 2 (25 chunks of 10x5 queries): score matmuls straight off kq_all
strided window APs; NEG mask added into PSUM by matmul; exp w/ scale.
Phase 3 (10 chunks): output projection; host rescales.
"""

import math
import os
import sys

import numpy as np

sys.path.insert(0, "/opt/trn_rl_repo")

import ml_dtypes  # noqa: E402

import concourse.bass as bass  # noqa: E402
import concourse.bacc as bacc  # noqa: E402
import concourse.mybir as mybir  # noqa: E402
import concourse.tile as tile  # noqa: E402

F32 = mybir.dt.float32
FP8 = mybir.dt.float8e4
BF16 = mybir.dt.bfloat16
AX = mybir.AxisListType
ALU = mybir.AluOpType
ACTF = mybir.ActivationFunctionType
DR = mybir.MatmulPerfMode.DoubleRow

# Problem constants
N_FULL = 10000
GRID = 100
T = 5
C = 256
NH = 8
DK = 32
CORES = 8
CR, CC_ = 2, 4             # core grid 2 x 4
BR, BC = 50, 25            # block rows/cols per core
NLOC = BR * BC             # 1250 own pixels per core
HR, HC = BR + 4, BC + 4    # 54 x 29 local region (with halo)
NH_PIX = HR * HC           # 1566
NP = 128
G = (NH_PIX + NP - 1) // NP        # 13 chunks
NPAD = G * NP                      # 1664
GO = 10                            # output-projection chunks
NPO = NLOC // GO                   # 125
SQR, SQC = 10, 5                   # query block 10 x 5
NS_R, NS_C = BR // SQR, BC // SQC  # 5 x 5 = 25 spatial chunks
NS = NS_R * NS_C
NQ = SQR * SQC                     # 50
WR, WC = SQR + 4, SQC + 4          # 14 x 9 window
NW = WR * WC                       # 126
NEGM = -1e9

_CACHE = {}


def _bf16(a):
    return np.asarray(a, dtype=ml_dtypes.bfloat16)


def _fp8(a):
    return np.asarray(a, dtype=ml_dtypes.float8_e4m3)


def _pow2_scale(w, target=8.0):
    rms = float(np.sqrt(np.mean(np.asarray(w, np.float64) ** 2)))
    s = 2.0 ** round(math.log2(target / max(rms, 1e-30)))
    assert float(np.abs(w).max()) * s < 350.0, "fp8 overflow risk"
    return s


def _build_graph():
    nc = bacc.Bacc(
        "TRN2",
        target_bir_lowering=False,
        debug=False,
        enable_asserts=False,
        num_devices=CORES,
    )

    # ---------------- I/O ----------------
    x_d = nc.dram_tensor("x", [G, 128, 2 * T * NP], FP8, kind="ExternalInput")
    wt_d = nc.dram_tensor("wt", [2, 128, 3 * C], FP8, kind="ExternalInput")
    we_d = nc.dram_tensor("we", [2, 128, 3 * C], FP8, kind="ExternalInput")
    wo_d = nc.dram_tensor("wo", [2, 128, C], FP8, kind="ExternalInput")
    bqt_d = nc.dram_tensor("bqt", [1, C], BF16, kind="ExternalInput")
    bqe_d = nc.dram_tensor("bqe", [1, C], BF16, kind="ExternalInput")
    boe_d = nc.dram_tensor("boe", [1, C], BF16, kind="ExternalInput")
    bo4_d = nc.dram_tensor("bo4", [128, 4], BF16, kind="ExternalInput")
    ident_d = nc.dram_tensor("ident", [128, 128], BF16, kind="ExternalInput")
    ones1_d = nc.dram_tensor("ones1", [1, 128], BF16, kind="ExternalInput")
    masks_d = nc.dram_tensor("masks", [NW, NS * NQ], BF16, kind="ExternalInput")
    scal_d = nc.dram_tensor("scal", [128, 2], F32, kind="ExternalInput")
    out_d = nc.dram_tensor("out", [NLOC, C], BF16, kind="ExternalOutput")

    with tile.TileContext(nc) as tc:
        with (
            tc.tile_pool(name="const", bufs=1) as cpool,
            tc.tile_pool(name="dram", bufs=1, space="DRAM") as dpool,
            tc.tile_pool(name="sb", bufs=2) as sb,
            tc.tile_pool(name="kp", bufs=1, space="PSUM") as pk,
            tc.tile_pool(name="qs", bufs=1, space="PSUM") as pq,
            tc.tile_pool(name="vp", bufs=2, space="PSUM") as pv,
            tc.tile_pool(name="ep", bufs=1, space="PSUM") as pe,
            tc.tile_pool(name="pvE", bufs=1, space="PSUM") as pvE,
            tc.tile_pool(name="scx", bufs=2, space="PSUM") as px2,
            tc.tile_pool(name="vwp", bufs=6) as vwp,
        ):
            v_dram = dpool.tile([NPAD, C], BF16, tag="v_dram", name="v_dram")

            # ---------- constants ----------
            # per-projection weight tiles: k-tile pitch must equal the
            # moving free size for DoubleRow (contiguous [2, 256] runs)
            wt3, we3 = [], []
            for j in range(3):
                t_ = cpool.tile([128, 2, C], FP8, tag=f"wt{j}")
                nc.sync.dma_start(
                    t_[:], wt_d.ap()[:, :, C * j : C * (j + 1)].rearrange(
                        "a p c -> p a c")
                )
                wt3.append(t_)
                e_ = cpool.tile([128, 2, C], FP8, tag=f"we{j}")
                nc.sync.dma_start(
                    e_[:], we_d.ap()[:, :, C * j : C * (j + 1)].rearrange(
                        "a p c -> p a c")
                )
                we3.append(e_)
            wtk, wtv, wtq = wt3
            wek, weq, wev = we3
            wo = cpool.tile([128, 2, C], FP8, tag="wo")
            nc.sync.dma_start(wo[:], wo_d.ap().rearrange("a p c -> p a c"))
            bqt = cpool.tile([1, C], BF16, tag="bqt")
            nc.sync.dma_start(bqt[:], bqt_d.ap())
            bqe = cpool.tile([1, C], BF16, tag="bqe")
            nc.sync.dma_start(bqe[:], bqe_d.ap())
            boe = cpool.tile([1, C], BF16, tag="boe")
            nc.sync.dma_start(boe[:], boe_d.ap())
            bo4 = cpool.tile([128, 4], BF16, tag="bo4")
            nc.sync.dma_start(bo4[:], bo4_d.ap())
            ident = cpool.tile([128, 128], BF16, tag="ident")
            nc.sync.dma_start(ident[:], ident_d.ap())
            ones1 = cpool.tile([1, 128], BF16, tag="ones1")
            nc.sync.dma_start(ones1[:], ones1_d.ap())
            masks = cpool.tile([128, NS, NQ], BF16, tag="masks")
            nc.sync.dma_start(
                masks[0:NW, :, :], masks_d.ap().rearrange("w (s q) -> w s q", s=NS)
            )
            onesw = cpool.tile([128, 1], BF16, tag="onesw")
            nc.vector.memset(onesw[:], 1.0)
            # runtime f32 scales (per-partition replicated):
            # col 0 = temporal exp scale, col 1 = spatial exp scale
            scal = cpool.tile([128, 2], F32, tag="scal")
            nc.sync.dma_start(scal[:], scal_d.ap())

            kq_all = cpool.tile([128, 2, 2, NPAD], BF16, tag="kq_all")
            cT_all = cpool.tile([128, 2, NLOC], FP8, tag="cT_all")

            kqv = kq_all[:, :, :, 0:NH_PIX].rearrange(
                "p a b (r c) -> p a b r c", r=HR
            )
            vdv = v_dram[0:NH_PIX, :].rearrange("(r c) x -> r c x", r=HR)
            cTv = cT_all[:].rearrange("p a (r c) -> p a r c", r=BR)

            PH = os.environ.get("KERNEL_PHASES", "123")
            H = {}   # per-chunk tile handles passed from stage A to stage B

            def emit_A1(g):
                """temporal projections + prods for chunk g"""
                xg = sb.tile([128, 2, T, NP], FP8, tag="xg")
                nc.sync.dma_start(
                    xg[:], x_d.ap()[g].rearrange("p (a t n) -> p a t n", a=2, t=T)
                )
                # qT ch-major + bias; shares its PSUM bank with the score
                # accumulator s (disjoint byte ranges)
                qs_t = pq.tile([128, 296], F32, tag="qs", name=f"qs{g}")
                qp = qs_t[:, 0:256].rearrange("p (a n) -> p a n", a=2)
                st = qs_t[:, 256 : 256 + T * NH].rearrange(
                    "p (t h) -> p t h", t=T
                )
                for gq in range(2):
                    for cc in range(2):
                        nc.tensor.matmul(
                            qp[:, gq, :],
                            wtq[:, cc, 128 * gq : 128 * gq + 128],
                            xg[:, cc, T - 1, :],
                            start=(cc == 0), stop=False,
                            skip_group_check=True,
                        )
                    nc.tensor.matmul(
                        qp[:, gq, :],
                        bqt[0:1, 128 * gq : 128 * gq + 128],
                        ones1[0:1, 0:NP],
                        start=False, stop=True, skip_group_check=True,
                    )
                # hw: a DVE op may read at most ONE input from PSUM, so qT
                # moves to SBUF before the prod muls
                q_sb = sb.tile([128, 2, NP], BF16, tag="q_sb")
                if g % 2 == 0:
                    nc.scalar.copy(q_sb[:], qp[:])
                else:
                    nc.vector.tensor_copy(q_sb[:], qp[:])
                # kT ch-major in t-pairs; prod muls consume them right away.
                # All prods run before any s-matmul (whose start=True
                # pending-zeroes the whole shared bank in the sim, so qp must
                # be fully consumed first).
                prods = []
                for (t0, nt) in ((0, 2), (2, 2), (4, 1)):
                    kp_t = pk.tile([128, 2, 2, NP], F32, tag="kp",
                                   name=f"k{g}_{t0}")
                    for gk in range(2):
                        for cc in range(2):
                            nc.tensor.matmul(
                                kp_t[:, gk, 0:nt, :],
                                wtk[:, cc, 128 * gk : 128 * gk + 128],
                                xg[:, cc, t0 : t0 + nt, :],
                                start=(cc == 0), stop=(cc == 1),
                            )
                    prod = sb.tile([128, 2, 2, NP], BF16, tag=f"prod{t0}",
                                   name=f"prod{g}_{t0}")
                    nc.vector.tensor_mul(
                        prod[:, :, 0:nt, :],
                        kp_t[:, :, 0:nt, :],
                        q_sb[:].unsqueeze(2).broadcast_to((128, 2, nt, NP)),
                    )
                    prods.append((t0, nt, prod))
                # vPM pixel-major [px, t, (d h)] in t-pairs
                v_sb = sb.tile([128, T, C], BF16, tag="v_sb")
                for (t0, nt) in ((0, 2), (2, 2), (4, 1)):
                    vp_t = pv.tile([128, 512], F32, tag="vp",
                                   name=f"v{g}_{t0}")[:, 0 : nt * C].rearrange(
                        "p (a c) -> p a c", a=nt
                    )
                    for dt_ in range(nt):
                        for cc in range(2):
                            nc.tensor.matmul(
                                vp_t[:, dt_, :],
                                xg[:, cc, t0 + dt_, :],
                                wtv[:, cc, :],
                                start=(cc == 0), stop=(cc == 1),
                            )
                    nc.scalar.copy(v_sb[:, t0 : t0 + nt, :], vp_t[:])
                H[g] = {"prods": prods, "st": st, "v_sb": v_sb}

            def emit_A2(g):
                """scores + softmax for chunk g"""
                st = H[g]["st"]
                for (t0, nt, prod) in H[g].pop("prods"):
                    for gk in range(2):
                        for dt_ in range(nt):
                            nc.tensor.matmul(
                                st[:, t0 + dt_, 4 * gk : 4 * gk + 4],
                                prod[:, gk, dt_, :],
                                bo4[:, :],
                                start=True, stop=True, skip_group_check=True,
                            )
                es = sb.tile([128, T, NH], BF16, tag="es")
                nc.scalar.activation(es[:], st[:], ACTF.Exp,
                                     scale=scal[:, 0:1])
                tsum = sb.tile([128, NH], F32, tag="tsum")
                nc.vector.tensor_reduce(
                    tsum[:], es[:].rearrange("p t h -> p h t"), axis=AX.X,
                    op=ALU.add,
                )
                rinv = sb.tile([128, NH], F32, tag="rinv")
                nc.vector.reciprocal(rinv[:], tsum[:])
                p_t = sb.tile([128, T, NH], BF16, tag="p_t")
                nc.gpsimd.tensor_mul(
                    p_t[:], es[:], rinv[:].unsqueeze(1).broadcast_to((128, T, NH))
                )
                H[g]["p_t"] = p_t

            def emit_B1(g):
                """ctx accumulation (Pool) for chunk g"""
                p_t, v_sb = H[g]["p_t"], H[g]["v_sb"]
                wv = sb.tile([128, T, DK, NH], BF16, tag="wv")
                nc.gpsimd.tensor_mul(
                    wv[:],
                    v_sb[:].rearrange("p t (d h) -> p t d h", d=DK),
                    p_t[:].unsqueeze(2).broadcast_to((128, T, DK, NH)),
                )
                c1 = sb.tile([128, 2, C], BF16, tag="c1")
                nc.gpsimd.tensor_add(
                    c1[:].rearrange("p a (d h) -> p a d h", d=DK),
                    wv[:, 0:2], wv[:, 2:4],
                )
                c2 = sb.tile([128, C], BF16, tag="c2")
                nc.gpsimd.tensor_add(c2[:], c1[:, 0, :], c1[:, 1, :])
                ctx = sb.tile([128, C], BF16, tag="ctx")
                nc.gpsimd.tensor_add(
                    ctx[:].rearrange("p (d h) -> p d h", d=DK),
                    c2[:].rearrange("p (d h) -> p d h", d=DK),
                    wv[:, 4],
                )
                # ctxT via DMA transpose (no PSUM), then fp8 convert on Pool
                ctb = sb.tile([128, 2, NP], BF16, tag="ctb")
                nc.sync.dma_start_transpose(ctb[:], ctx[:])
                ctxT = sb.tile([128, 2, NP], FP8, tag="ctxT")
                nc.gpsimd.tensor_copy(ctxT[:], ctb[:])
                H[g]["ctxT"] = ctxT

            def emit_B2(g):
                """spatial projections for chunk g"""
                ctxT = H[g]["ctxT"]

                # spatial projections: kqE [128, {k,q}, grp, px]
                kq_t = pe.tile([128, 512], F32, tag="ep", name=f"kq{g}")
                kqE = kq_t[:].rearrange("p (a b n) -> p a b n", a=2, b=2)
                last = g == G - 1   # chunk 12 has no query pixels
                for gk in range(2):
                    for cc in range(2):
                        nc.tensor.matmul(
                            kqE[:, 0, gk, :],
                            wek[:, cc, 128 * gk : 128 * gk + 128],
                            ctxT[:, cc, :],
                            start=(cc == 0), stop=(cc == 1),
                        )
                if not last:
                    for gk in range(2):
                        for cc in range(2):
                            nc.tensor.matmul(
                                kqE[:, 1, gk, :],
                                weq[:, cc, 128 * gk : 128 * gk + 128],
                                ctxT[:, cc, :],
                                start=(cc == 0), stop=False,
                                skip_group_check=True,
                            )
                        nc.tensor.matmul(
                            kqE[:, 1, gk, :],
                            bqe[0:1, 128 * gk : 128 * gk + 128],
                            ones1[0:1, 0:NP],
                            start=False, stop=True, skip_group_check=True,
                        )
                nkq = 1 if last else 2
                dst = kq_all[:, 0:nkq, :, g * NP : (g + 1) * NP]
                if g % 2 == 0:
                    nc.scalar.copy(dst, kqE[:, 0:nkq, :, :])
                else:
                    nc.vector.tensor_copy(dst, kqE[:, 0:nkq, :, :])

                # spatial v pixel-major
                vE = pvE.tile([128, C], F32, tag="pvE", name=f"vE{g}")
                for cc in range(2):
                    nc.tensor.matmul(
                        vE[:],
                        ctxT[:, cc, :],
                        wev[:, cc, :],
                        start=(cc == 0), stop=(cc == 1),
                    )
                v1 = sb.tile([128, C], BF16, tag="v1")
                if g % 2 == 0:
                    nc.vector.tensor_copy(v1[:], vE[:])
                else:
                    nc.scalar.copy(v1[:], vE[:])
                nc.sync.dma_start(v_dram[g * NP : (g + 1) * NP, :], v1[:])
                del H[g]

            VW = {}

            def emit_P2pre(s):
                r0, c0 = (s // NS_C) * SQR, (s % NS_C) * SQC
                vw = vwp.tile([128, C], BF16, tag="vw", name=f"vw{s}")
                qdma = nc.sync if s % 2 == 0 else nc.gpsimd
                qdma.dma_start(
                    vw[0:NW, :], vdv[r0 : r0 + WR, c0 : c0 + WC, :]
                )
                VW[s] = vw

            def emit_P2(s):
                r0, c0 = (s // NS_C) * SQR, (s % NS_C) * SQC
                vw = VW.pop(s)
                # hw matmul operand APs must have a single free dim:
                # materialize the strided k/q windows contiguously (Pool)
                qw = sb.tile([128, 2, NQ], BF16, tag="qw")
                nc.gpsimd.tensor_copy(
                    qw[:], kqv[:, 1, :, 2 + r0 : 2 + r0 + SQR,
                               2 + c0 : 2 + c0 + SQC]
                )
                kw = sb.tile([128, 2, NW], BF16, tag="kw")
                nc.gpsimd.tensor_copy(
                    kw[:], kqv[:, 0, :, r0 : r0 + WR, c0 : c0 + WC]
                )
                sc_t = px2.tile([128, 512], F32, tag="scx", name=f"sc{s}")
                sc = sc_t[0:NW, 0 : NH * NQ].rearrange("p (h q) -> p h q", h=NH)
                for h in range(NH):
                    gk, mk = h // 4, h % 4
                    nc.tensor.matmul(
                        sc[:, h, :],
                        kw[32 * mk : 32 * mk + 32, gk, :],
                        qw[32 * mk : 32 * mk + 32, gk, :],
                        start=True, stop=False, skip_group_check=True,
                        tile_position=(32 * mk, 0),
                    )
                    nc.tensor.matmul(
                        sc[:, h, :],
                        ident[0:NW, 0:NW],
                        masks[0:NW, s, :],
                        start=False, stop=True, skip_group_check=True,
                    )
                E = sb.tile([128, NH, NQ], BF16, tag="E")
                nc.scalar.activation(E[0:NW, :, :], sc[:], ACTF.Exp,
                                     scale=scal[0:NW, 1:2])
                cx_t = px2.tile([128, 512], F32, tag="scx", name=f"cx{s}")
                cx = cx_t[0:NQ, 0 : NH * (DK + 1)].rearrange(
                    "p (h d) -> p h d", h=NH
                )
                for h in range(NH):
                    nc.tensor.matmul(
                        cx[:, h, 0:DK],
                        E[0:NW, h, :],
                        vw[0:NW, DK * h : DK * h + DK],
                        start=True, stop=True,
                    )
                    nc.tensor.matmul(
                        cx[:, h, DK : DK + 1],
                        E[0:NW, h, :],
                        onesw[0:NW, :],
                        start=True, stop=True,
                    )
                srinv = sb.tile([NQ, NH], F32, tag="srinv")
                nc.vector.reciprocal(srinv[:], cx[:, :, DK])
                ctxn = sb.tile([64, C], BF16, tag="ctxn")
                # rows 50:64 are transpose pad (never consumed downstream);
                # memset from 32 (engines need 32-aligned start partitions),
                # the overlap is overwritten by the normalize below
                nc.gpsimd.memset(ctxn[32:64, :], 0.0)
                nc.vector.tensor_mul(
                    ctxn[0:NQ, :].rearrange("q (h d) -> q h d", h=NH),
                    cx[:, :, 0:DK],
                    srinv[:].unsqueeze(2).broadcast_to((NQ, NH, DK)),
                )
                ntpT = sb.tile([128, 2, 64], BF16, tag="ntpT")
                nc.sync.dma_start_transpose(ntpT[:], ctxn[:])
                csel = cTv[:, :, r0 : r0 + SQR, c0 : c0 + SQC]
                nc.gpsimd.tensor_copy(
                    csel, ntpT[:, :, 0:NQ].rearrange(
                        "p a (r c) -> p a r c", r=SQR
                    ),
                )

            def emit_P3(g):
                op = pe.tile([128, 512], F32, tag="ep", name=f"op{g}")[
                    0:NPO, 0:C
                ]
                for cc in range(2):
                    nc.tensor.matmul(
                        op[:],
                        cT_all[:, cc, g * NPO : g * NPO + NPO],
                        wo[:, cc, :],
                        start=(cc == 0), stop=False,
                        skip_group_check=True,
                    )
                nc.tensor.matmul(
                    op[:], ones1[0:1, 0:NPO], boe[0:1, 0:C],
                    start=False, stop=True, skip_group_check=True,
                )
                o_sb = sb.tile([NPO, C], BF16, tag="o_sb")
                if g % 2 == 0:
                    nc.scalar.copy(o_sb[:], op[:])
                else:
                    nc.vector.tensor_copy(o_sb[:], op[:])
                nc.sync.dma_start(out_d.ap()[g * NPO : (g + 1) * NPO, :], o_sb[:])

            # ---- interleaved emission schedule (software pipelining) ----
            # A1(g) -> [B(g-1)] -> A2(g), with phase-2 chunks emitted as soon
            # as their window rows are fully written, and phase-3 chunks as
            # their cT bands complete.
            p2q = list(range(NS)) if "2" in PH else []
            p3q = list(range(GO)) if "3" in PH else []
            p2_done = 0

            def p2_ready(s, g_written):
                rs = (s // NS_C) * SQR
                return (rs + WR) * HC <= g_written * NP

            def p3_ready(g3, n_p2_done):
                return (g3 // 2 + 1) * NS_C <= n_p2_done

            sched_gs = list(range(G)) if "1" in PH else []
            p2pre = list(range(NS)) if "2" in PH else []
            for gi, g in enumerate(sched_gs + [None]):
                if g is not None:
                    emit_A1(g)
                if gi >= 1 and "1" in PH:
                    emit_B1(sched_gs[gi - 1])
                if g is not None:
                    emit_A2(g)
                if gi >= 1 and "1" in PH:
                    emit_B2(sched_gs[gi - 1])
                g_written = gi if "1" in PH else G
                while p2pre and p2_ready(p2pre[0], g_written):
                    emit_P2pre(p2pre.pop(0))
                while p2q and p2q[0] in VW and (len(VW) >= 4 or not p2pre):
                    emit_P2(p2q.pop(0))
                    p2_done += 1
                    while p3q and (p3q[0] // 2 + 1) * NS_C + 3 <= p2_done:
                        emit_P3(p3q.pop(0))
            while p2pre:
                emit_P2pre(p2pre.pop(0))
            while p2q:
                emit_P2(p2q.pop(0))
                p2_done += 1
                while p3q and (p3q[0] // 2 + 1) * NS_C + 3 <= p2_done:
                    emit_P3(p3q.pop(0))
            for g3 in p3q:
                emit_P3(g3)
            if "3" not in PH:
                zz = sb.tile([NPO, C], BF16, tag="o_sb", name="zz")
                nc.vector.memset(zz[:], 0.0)
                nc.sync.dma_start(out_d.ap()[0:NPO, :], zz[:])

    nc.compile()
    return nc


# revision 8
# speedup vs baseline: 1.0776x; 1.0757x over previous
"""Trainium2 distributed kernel for nn_AttentionFusion — v2.

Channel-major temporal attention, fp8 DoubleRow matmuls, PE score
reduction via block-ones matmuls, spatial mask folded into score PSUM,
direct strided-AP windows (no head-regroup scatter).

Per core: 2x4 grid of 50x25 blocks + 2-px halo (54x29 local = 1566 px,
padded to 13 chunks of 128).

Phase 1 (13 chunks): kT/qT ch-major + vPM pixel-major via fp8 DoubleRow;
scores = blockones matmuls over DVE products; softmax pixel-major;
ctx = p-weighted v (Pool); spatial projections from fp8 ctxT.
Phase 2 (25 chunks of 10x5 queries): score matmuls straight off kq_all
strided window APs; NEG mask added into PSUM by matmul; exp w/ scale.
Phase 3 (10 chunks): output projection; host rescales.
"""

import math
import os
import sys

import numpy as np

sys.path.insert(0, "/opt/trn_rl_repo")

import ml_dtypes  # noqa: E402

import concourse.bass as bass  # noqa: E402
import concourse.bacc as bacc  # noqa: E402
import concourse.mybir as mybir  # noqa: E402
import concourse.tile as tile  # noqa: E402

F32 = mybir.dt.float32
FP8 = mybir.dt.float8e4
BF16 = mybir.dt.bfloat16
AX = mybir.AxisListType
ALU = mybir.AluOpType
ACTF = mybir.ActivationFunctionType
DR = mybir.MatmulPerfMode.DoubleRow

# Problem constants
N_FULL = 10000
GRID = 100
T = 5
C = 256
NH = 8
DK = 32
CORES = 8
CR, CC_ = 2, 4             # core grid 2 x 4
BR, BC = 50, 25            # block rows/cols per core
NLOC = BR * BC             # 1250 own pixels per core
HR, HC = BR + 4, BC + 4    # 54 x 29 local region (with halo)
NH_PIX = HR * HC           # 1566
NP = 128
G = (NH_PIX + NP - 1) // NP        # 13 chunks
NPAD = G * NP                      # 1664
GO = 10                            # output-projection chunks
NPO = NLOC // GO                   # 125
SQR, SQC = 10, 5                   # query block 10 x 5
NS_R, NS_C = BR // SQR, BC // SQC  # 5 x 5 = 25 spatial chunks
NS = NS_R * NS_C
NQ = SQR * SQC                     # 50
WR, WC = SQR + 4, SQC + 4          # 14 x 9 window
NW = WR * WC                       # 126
NEGM = -1e9

_CACHE = {}


def _bf16(a):
    return np.asarray(a, dtype=ml_dtypes.bfloat16)


def _fp8(a):
    return np.asarray(a, dtype=ml_dtypes.float8_e4m3)


def _pow2_scale(w, target=8.0):
    rms = float(np.sqrt(np.mean(np.asarray(w, np.float64) ** 2)))
    s = 2.0 ** round(math.log2(target / max(rms, 1e-30)))
    assert float(np.abs(w).max()) * s < 350.0, "fp8 overflow risk"
    return s


def _build_graph():
    nc = bacc.Bacc(
        "TRN2",
        target_bir_lowering=False,
        debug=False,
        enable_asserts=False,
        num_devices=CORES,
    )

    # ---------------- I/O ----------------
    x_d = nc.dram_tensor("x", [G, 128, 2 * T * NP], FP8, kind="ExternalInput")
    wt_d = nc.dram_tensor("wt", [2, 128, 3 * C], FP8, kind="ExternalInput")
    we_d = nc.dram_tensor("we", [2, 128, 3 * C], FP8, kind="ExternalInput")
    wo_d = nc.dram_tensor("wo", [2, 128, C], FP8, kind="ExternalInput")
    bqt_d = nc.dram_tensor("bqt", [1, C], BF16, kind="ExternalInput")
    bqe_d = nc.dram_tensor("bqe", [1, C], BF16, kind="ExternalInput")
    boe_d = nc.dram_tensor("boe", [1, C], BF16, kind="ExternalInput")
    bo4_d = nc.dram_tensor("bo4", [128, 4], BF16, kind="ExternalInput")
    ident_d = nc.dram_tensor("ident", [128, 128], BF16, kind="ExternalInput")
    ones1_d = nc.dram_tensor("ones1", [1, 128], BF16, kind="ExternalInput")
    masks_d = nc.dram_tensor("masks", [NW, NS * NQ], BF16, kind="ExternalInput")
    scal_d = nc.dram_tensor("scal", [128, 2], F32, kind="ExternalInput")
    out_d = nc.dram_tensor("out", [NLOC, C], BF16, kind="ExternalOutput")

    with tile.TileContext(nc) as tc:
        with (
            tc.tile_pool(name="const", bufs=1) as cpool,
            tc.tile_pool(name="dram", bufs=1, space="DRAM") as dpool,
            tc.tile_pool(name="sb", bufs=2) as sb,
            tc.tile_pool(name="kp", bufs=2, space="PSUM") as pk,
            tc.tile_pool(name="qs", bufs=1, space="PSUM") as pq,
            tc.tile_pool(name="vp", bufs=2, space="PSUM") as pv,
            tc.tile_pool(name="ep", bufs=1, space="PSUM") as pe,
            tc.tile_pool(name="scx", bufs=2, space="PSUM") as px2,
            tc.tile_pool(name="vwp", bufs=6) as vwp,
        ):
            v_dram = dpool.tile([NPAD, C], BF16, tag="v_dram", name="v_dram")

            # ---------- constants ----------
            # per-projection weight tiles: k-tile pitch must equal the
            # moving free size for DoubleRow (contiguous [2, 256] runs)
            wt3, we3 = [], []
            for j in range(3):
                t_ = cpool.tile([128, 2, C], FP8, tag=f"wt{j}")
                nc.sync.dma_start(
                    t_[:], wt_d.ap()[:, :, C * j : C * (j + 1)].rearrange(
                        "a p c -> p a c")
                )
                wt3.append(t_)
                e_ = cpool.tile([128, 2, C], FP8, tag=f"we{j}")
                nc.sync.dma_start(
                    e_[:], we_d.ap()[:, :, C * j : C * (j + 1)].rearrange(
                        "a p c -> p a c")
                )
                we3.append(e_)
            wtk, wtv, wtq = wt3
            wek, weq, wev = we3
            wo = cpool.tile([128, 2, C], FP8, tag="wo")
            nc.sync.dma_start(wo[:], wo_d.ap().rearrange("a p c -> p a c"))
            bqt = cpool.tile([1, C], BF16, tag="bqt")
            nc.sync.dma_start(bqt[:], bqt_d.ap())
            bqe = cpool.tile([1, C], BF16, tag="bqe")
            nc.sync.dma_start(bqe[:], bqe_d.ap())
            boe = cpool.tile([1, C], BF16, tag="boe")
            nc.sync.dma_start(boe[:], boe_d.ap())
            bo4 = cpool.tile([128, 4], BF16, tag="bo4")
            nc.sync.dma_start(bo4[:], bo4_d.ap())
            ident = cpool.tile([128, 128], BF16, tag="ident")
            nc.sync.dma_start(ident[:], ident_d.ap())
            ones1 = cpool.tile([1, 128], BF16, tag="ones1")
            nc.sync.dma_start(ones1[:], ones1_d.ap())
            masks = cpool.tile([128, NS, NQ], BF16, tag="masks")
            nc.sync.dma_start(
                masks[0:NW, :, :], masks_d.ap().rearrange("w (s q) -> w s q", s=NS)
            )
            onesw = cpool.tile([128, 1], BF16, tag="onesw")
            nc.vector.memset(onesw[:], 1.0)
            # runtime f32 scales (per-partition replicated):
            # col 0 = temporal exp scale, col 1 = spatial exp scale
            scal = cpool.tile([128, 2], F32, tag="scal")
            nc.sync.dma_start(scal[:], scal_d.ap())

            kq_all = cpool.tile([128, 2, 2, NPAD], BF16, tag="kq_all")
            cT_all = cpool.tile([128, 2, NLOC], BF16, tag="cT_all")

            kqv = kq_all[:, :, :, 0:NH_PIX].rearrange(
                "p a b (r c) -> p a b r c", r=HR
            )
            vdv = v_dram[0:NH_PIX, :].rearrange("(r c) x -> r c x", r=HR)
            cTv = cT_all[:].rearrange("p a (r c) -> p a r c", r=BR)

            PH = os.environ.get("KERNEL_PHASES", "123")
            H = {}   # per-chunk tile handles passed from stage A to stage B

            def emit_A1(g):
                """temporal projections + prods for chunk g"""
                xg = sb.tile([128, 2, T, NP], FP8, tag="xg")
                nc.sync.dma_start(
                    xg[:], x_d.ap()[g].rearrange("p (a t n) -> p a t n", a=2, t=T)
                )
                # qT ch-major + bias; shares its PSUM bank with the score
                # accumulator s (disjoint byte ranges)
                qs_t = pq.tile([128, 296], F32, tag="qs", name=f"qs{g}")
                qp = qs_t[:, 0:256].rearrange("p (a n) -> p a n", a=2)
                st = qs_t[:, 256 : 256 + T * NH].rearrange(
                    "p (t h) -> p t h", t=T
                )
                for gq in range(2):
                    for cc in range(2):
                        nc.tensor.matmul(
                            qp[:, gq, :],
                            wtq[:, cc, 128 * gq : 128 * gq + 128],
                            xg[:, cc, T - 1, :],
                            start=(cc == 0), stop=False,
                            skip_group_check=True,
                        )
                    nc.tensor.matmul(
                        qp[:, gq, :],
                        bqt[0:1, 128 * gq : 128 * gq + 128],
                        ones1[0:1, 0:NP],
                        start=False, stop=True, skip_group_check=True,
                    )
                # hw: a DVE op may read at most ONE input from PSUM, so qT
                # moves to SBUF before the prod muls
                q_sb = sb.tile([128, 2, NP], BF16, tag="q_sb")
                if g % 2 == 0:
                    nc.scalar.copy(q_sb[:], qp[:])
                else:
                    nc.vector.tensor_copy(q_sb[:], qp[:])
                # kT ch-major in t-pairs; prod muls consume them right away.
                # All prods run before any s-matmul (whose start=True
                # pending-zeroes the whole shared bank in the sim, so qp must
                # be fully consumed first).
                prods = []
                for (t0, nt) in ((0, 2), (2, 2), (4, 1)):
                    kp_t = pk.tile([128, 2, 2, NP], F32, tag="kp",
                                   name=f"k{g}_{t0}")
                    for gk in range(2):
                        for cc in range(2):
                            nc.tensor.matmul(
                                kp_t[:, gk, 0:nt, :],
                                wtk[:, cc, 128 * gk : 128 * gk + 128],
                                xg[:, cc, t0 : t0 + nt, :],
                                start=(cc == 0), stop=(cc == 1),
                            )
                    prod = sb.tile([128, 2, 2, NP], BF16, tag=f"prod{t0}",
                                   name=f"prod{g}_{t0}")
                    nc.vector.tensor_mul(
                        prod[:, :, 0:nt, :],
                        kp_t[:, :, 0:nt, :],
                        q_sb[:].unsqueeze(2).broadcast_to((128, 2, nt, NP)),
                    )
                    prods.append((t0, nt, prod))
                # vPM pixel-major [px, t, (d h)] in t-pairs
                v_sb = sb.tile([128, T, C], BF16, tag="v_sb")
                for (t0, nt) in ((0, 2), (2, 2), (4, 1)):
                    vp_t = pv.tile([128, 512], F32, tag="vp",
                                   name=f"v{g}_{t0}")[:, 0 : nt * C].rearrange(
                        "p (a c) -> p a c", a=nt
                    )
                    for dt_ in range(nt):
                        for cc in range(2):
                            nc.tensor.matmul(
                                vp_t[:, dt_, :],
                                xg[:, cc, t0 + dt_, :],
                                wtv[:, cc, :],
                                start=(cc == 0), stop=(cc == 1),
                            )
                    nc.scalar.copy(v_sb[:, t0 : t0 + nt, :], vp_t[:])
                H[g] = {"prods": prods, "st": st, "v_sb": v_sb}

            def emit_A2(g):
                """scores + softmax for chunk g"""
                st = H[g]["st"]
                for (t0, nt, prod) in H[g].pop("prods"):
                    for gk in range(2):
                        for dt_ in range(nt):
                            nc.tensor.matmul(
                                st[:, t0 + dt_, 4 * gk : 4 * gk + 4],
                                prod[:, gk, dt_, :],
                                bo4[:, :],
                                start=True, stop=True, skip_group_check=True,
                            )
                es = sb.tile([128, T, NH], BF16, tag="es")
                nc.scalar.activation(es[:], st[:], ACTF.Exp,
                                     scale=scal[:, 0:1])
                tsum = sb.tile([128, NH], F32, tag="tsum")
                nc.vector.tensor_reduce(
                    tsum[:], es[:].rearrange("p t h -> p h t"), axis=AX.X,
                    op=ALU.add,
                )
                rinv = sb.tile([128, NH], F32, tag="rinv")
                nc.vector.reciprocal(rinv[:], tsum[:])
                p_t = sb.tile([128, T, NH], BF16, tag="p_t")
                nc.gpsimd.tensor_mul(
                    p_t[:], es[:], rinv[:].unsqueeze(1).broadcast_to((128, T, NH))
                )
                H[g]["p_t"] = p_t

            def emit_B1(g):
                """ctx accumulation (Pool) for chunk g"""
                p_t, v_sb = H[g]["p_t"], H[g]["v_sb"]
                wv = sb.tile([128, T, DK, NH], BF16, tag="wv")
                nc.gpsimd.tensor_mul(
                    wv[:],
                    v_sb[:].rearrange("p t (d h) -> p t d h", d=DK),
                    p_t[:].unsqueeze(2).broadcast_to((128, T, DK, NH)),
                )
                c1 = sb.tile([128, 2, C], BF16, tag="c1")
                nc.gpsimd.tensor_add(
                    c1[:].rearrange("p a (d h) -> p a d h", d=DK),
                    wv[:, 0:2], wv[:, 2:4],
                )
                c2 = sb.tile([128, C], BF16, tag="c2")
                nc.gpsimd.tensor_add(c2[:], c1[:, 0, :], c1[:, 1, :])
                ctx = sb.tile([128, C], BF16, tag="ctx")
                nc.gpsimd.tensor_add(
                    ctx[:].rearrange("p (d h) -> p d h", d=DK),
                    c2[:].rearrange("p (d h) -> p d h", d=DK),
                    wv[:, 4],
                )
                # ctxT via DMA transpose (no PSUM); bf16 moving operand
                # costs the same as fp8 without DoubleRow
                ctxT = sb.tile([128, 2, NP], BF16, tag="ctxT")
                nc.sync.dma_start_transpose(ctxT[:], ctx[:])
                H[g]["ctxT"] = ctxT

            def emit_B2(g):
                """spatial projections for chunk g"""
                ctxT = H[g]["ctxT"]

                # spatial projections: kqE [128, {k,q}, grp, px]
                kq_t = pe.tile([128, 512], F32, tag="ep", name=f"kq{g}")
                kqE = kq_t[:].rearrange("p (a b n) -> p a b n", a=2, b=2)
                last = g == G - 1   # chunk 12 has no query pixels
                for gk in range(2):
                    for cc in range(2):
                        nc.tensor.matmul(
                            kqE[:, 0, gk, :],
                            wek[:, cc, 128 * gk : 128 * gk + 128],
                            ctxT[:, cc, :],
                            start=(cc == 0), stop=(cc == 1),
                        )
                if not last:
                    for gk in range(2):
                        for cc in range(2):
                            nc.tensor.matmul(
                                kqE[:, 1, gk, :],
                                weq[:, cc, 128 * gk : 128 * gk + 128],
                                ctxT[:, cc, :],
                                start=(cc == 0), stop=False,
                                skip_group_check=True,
                            )
                        nc.tensor.matmul(
                            kqE[:, 1, gk, :],
                            bqe[0:1, 128 * gk : 128 * gk + 128],
                            ones1[0:1, 0:NP],
                            start=False, stop=True, skip_group_check=True,
                        )
                nkq = 1 if last else 2
                dst = kq_all[:, 0:nkq, :, g * NP : (g + 1) * NP]
                if g % 2 == 0:
                    nc.scalar.copy(dst, kqE[:, 0:nkq, :, :])
                else:
                    nc.vector.tensor_copy(dst, kqE[:, 0:nkq, :, :])

                # spatial v pixel-major
                vE = pv.tile([128, 512], F32, tag="vp",
                             name=f"vE{g}")[:, 0:C]
                for cc in range(2):
                    nc.tensor.matmul(
                        vE[:],
                        ctxT[:, cc, :],
                        wev[:, cc, :],
                        start=(cc == 0), stop=(cc == 1),
                    )
                v1 = sb.tile([128, C], BF16, tag="v1")
                if g % 2 == 0:
                    nc.vector.tensor_copy(v1[:], vE[:])
                else:
                    nc.scalar.copy(v1[:], vE[:])
                nc.sync.dma_start(v_dram[g * NP : (g + 1) * NP, :], v1[:])
                del H[g]

            VW = {}
            KQB = {}

            def emit_band(b):
                """column-major contiguous k/q strips for query row band b"""
                rs = b * SQR
                kst = sb.tile([128, 2, HC, WR], BF16, tag="kst",
                              name=f"kst{b}")
                nc.gpsimd.tensor_copy(
                    kst[:],
                    kqv[:, 0, :, rs : rs + WR, :].rearrange(
                        "p g r c -> p g c r"),
                )
                qst = sb.tile([128, 2, BC, SQR], BF16, tag="qst",
                              name=f"qst{b}")
                nc.gpsimd.tensor_copy(
                    qst[:],
                    kqv[:, 1, :, 2 + rs : 2 + rs + SQR,
                        2 : 2 + BC].rearrange("p g r c -> p g c r"),
                )
                KQB[b] = (kst, qst)

            def emit_P2pre(s):
                if s // NS_C not in KQB:
                    emit_band(s // NS_C)
                r0, c0 = (s // NS_C) * SQR, (s % NS_C) * SQC
                vw = vwp.tile([128, C], BF16, tag="vw", name=f"vw{s}")
                qdma = nc.sync if s % 2 == 0 else nc.gpsimd
                qdma.dma_start(
                    vw[0:NW, :], vdv[r0 : r0 + WR, c0 : c0 + WC, :]
                )
                VW[s] = vw

            def emit_P2(s):
                r0, c0 = (s // NS_C) * SQR, (s % NS_C) * SQC
                vw = VW.pop(s)
                # hw matmul operands need one contiguous free dim: read
                # from the band's column-major strips (windows = contiguous
                # column ranges there). NOTE: scores come out (c-major) — the
                # w/q index order inside the matmul is (col, row); masks and
                # vw use the same (c, r) order (host side + vw gather below).
                kst, qst = KQB[s // NS_C]
                sc_t = px2.tile([128, 512], F32, tag="scx", name=f"sc{s}")
                sc = sc_t[0:NW, 0 : NH * NQ].rearrange("p (h q) -> p h q", h=NH)
                for h in range(NH):
                    gk, mk = h // 4, h % 4
                    nc.tensor.matmul(
                        sc[:, h, :],
                        kst[32 * mk : 32 * mk + 32, gk, c0 : c0 + WC, :],
                        qst[32 * mk : 32 * mk + 32, gk, c0 : c0 + SQC, :],
                        start=True, stop=False, skip_group_check=True,
                        tile_position=(32 * mk, 0),
                    )
                    nc.tensor.matmul(
                        sc[:, h, :],
                        ident[0:NW, 0:NW],
                        masks[0:NW, s, :],
                        start=False, stop=True, skip_group_check=True,
                    )
                E = sb.tile([128, NH, NQ], BF16, tag="E")
                nc.scalar.activation(E[0:NW, :, :], sc[:], ACTF.Exp,
                                     scale=scal[0:NW, 1:2])
                cx_t = px2.tile([128, 512], F32, tag="scx", name=f"cx{s}")
                cx = cx_t[0:NQ, 0 : NH * (DK + 1)].rearrange(
                    "p (h d) -> p h d", h=NH
                )
                for h in range(NH):
                    nc.tensor.matmul(
                        cx[:, h, 0:DK],
                        E[0:NW, h, :],
                        vw[0:NW, DK * h : DK * h + DK],
                        start=True, stop=True,
                    )
                    nc.tensor.matmul(
                        cx[:, h, DK : DK + 1],
                        E[0:NW, h, :],
                        onesw[0:NW, :],
                        start=True, stop=True,
                    )
                srinv = sb.tile([NQ, NH], F32, tag="srinv")
                nc.vector.reciprocal(srinv[:], cx[:, :, DK])
                ctxn = sb.tile([64, C], BF16, tag="ctxn")
                # rows 50:64 are transpose pad (never consumed downstream);
                # memset from 32 (engines need 32-aligned start partitions),
                # the overlap is overwritten by the normalize below
                nc.gpsimd.memset(ctxn[32:64, :], 0.0)
                nc.vector.tensor_mul(
                    ctxn[0:NQ, :].rearrange("q (h d) -> q h d", h=NH),
                    cx[:, :, 0:DK],
                    srinv[:].unsqueeze(2).broadcast_to((NQ, NH, DK)),
                )
                ntpT = sb.tile([128, 2, 64], BF16, tag="ntpT")
                nc.sync.dma_start_transpose(ntpT[:], ctxn[:])
                csel = cTv[:, :, r0 : r0 + SQR, c0 : c0 + SQC]
                nc.gpsimd.tensor_copy(
                    csel, ntpT[:, :, 0:NQ].rearrange(
                        "p a (r c) -> p a r c", r=SQR
                    ),
                )

            def emit_P3(g):
                op = pe.tile([128, 512], F32, tag="ep", name=f"op{g}")[
                    0:NPO, 0:C
                ]
                for cc in range(2):
                    nc.tensor.matmul(
                        op[:],
                        cT_all[:, cc, g * NPO : g * NPO + NPO],
                        wo[:, cc, :],
                        start=(cc == 0), stop=False,
                        skip_group_check=True,
                    )
                nc.tensor.matmul(
                    op[:], ones1[0:1, 0:NPO], boe[0:1, 0:C],
                    start=False, stop=True, skip_group_check=True,
                )
                o_sb = sb.tile([NPO, C], BF16, tag="o_sb")
                if g % 2 == 0:
                    nc.scalar.copy(o_sb[:], op[:])
                else:
                    nc.vector.tensor_copy(o_sb[:], op[:])
                nc.sync.dma_start(out_d.ap()[g * NPO : (g + 1) * NPO, :], o_sb[:])

            # ---- interleaved emission schedule (software pipelining) ----
            # A1(g) -> [B(g-1)] -> A2(g), with phase-2 chunks emitted as soon
            # as their window rows are fully written, and phase-3 chunks as
            # their cT bands complete.
            p2q = list(range(NS)) if "2" in PH else []
            p3q = list(range(GO)) if "3" in PH else []
            p2_done = 0

            def p2_ready(s, g_written):
                rs = (s // NS_C) * SQR
                return (rs + WR) * HC <= g_written * NP

            def p3_ready(g3, n_p2_done):
                return (g3 // 2 + 1) * NS_C <= n_p2_done

            sched_gs = list(range(G)) if "1" in PH else []
            p2pre = list(range(NS)) if "2" in PH else []
            for gi, g in enumerate(sched_gs + [None]):
                if g is not None:
                    emit_A1(g)
                if gi >= 1 and "1" in PH:
                    emit_B1(sched_gs[gi - 1])
                if g is not None:
                    emit_A2(g)
                if gi >= 1 and "1" in PH:
                    emit_B2(sched_gs[gi - 1])
                g_written = gi if "1" in PH else G
                while p2pre and p2_ready(p2pre[0], g_written):
                    emit_P2pre(p2pre.pop(0))
                while p2q and p2q[0] in VW and (len(VW) >= 4 or not p2pre):
                    emit_P2(p2q.pop(0))
                    p2_done += 1
                    while p3q and (p3q[0] // 2 + 1) * NS_C + 3 <= p2_done:
                        emit_P3(p3q.pop(0))
            while p2pre:
                emit_P2pre(p2pre.pop(0))
            while p2q:
                emit_P2(p2q.pop(0))
                p2_done += 1
                while p3q and (p3q[0] // 2 + 1) * NS_C + 3 <= p2_done:
                    emit_P3(p3q.pop(0))
            for g3 in p3q:
                emit_P3(g3)
            if "3" not in PH:
                zz = sb.tile([NPO, C], BF16, tag="o_sb", name="zz")
                nc.vector.memset(zz[:], 0.0)
                nc.sync.dma_start(out_d.ap()[0:NPO, :], zz[:])

    nc.compile()
    return nc
